# revision 44
# baseline (speedup 1.0000x reference)
# Trainium2 Bass kernel for nn_Attention_88313117540497.
#
# Reference computation (per batch b of 128):
#   v = x_b @ Wv; conv2d of each channel's 14x14 image with 27x27 qk at
#   padding 13; y = conv_out @ Wo + bo.
#
# Algebra:
#  1. The padded 27x27 conv on 14x14 covers every pixel pair, so it is a
#     dense 196x196 map M shared across batches/channels:
#         y_b = M @ x_b @ (Wv@Wo) + bo,   W = Wv@Wo (384x384).
#  2. PE mapping (out = lhsT.T @ rhs; lhsT stationary, rhs streams N
#     cycles):
#       stage A:  G^T_b = lhsT(X_b).T @ MT    X_b token-major, MT = M^T.
#                 3 d-chunks x 2 v-chunks, N=196.
#       stage B:  Y^T_b = lhsT(W).T @ G^T_b   3 e-chunks x 3 d-chunks,
#                 N=196, full 128x128 array occupancy (FLOP-optimal).
#     All operands bf16: halves DMA, enables Fast Weight Load so the
#     LDWEIGHTS stream hides under the matmul stream.
#  3. Software pipeline: stage A of batch b+1 issues before stage B of
#     batch b, covering the G-eviction latency; a gapless PE stream also
#     keeps the HAM clock gate at 8/8. N=392 bf16 warm-up matmuls ramp
#     HAM while the first x group is in flight.
#  4. x rides in a padded interleaved DRAM layout [128, b, 2, 384] so
#     every transfer is full-partition and contiguous per partition
#     (the natural 196-row layout leaves a slow 68-partition tail).
#  5. PSUM: four single-bank tiles per batch stage (g01/g2/y01/y2,
#     bufs=2 -> exactly 8 banks). The 01 tiles pack two 196-wide matmul
#     outputs contiguously inside one bank, so each eviction is one
#     contiguous 2D op; G and Y evictions are split scalar/vector on
#     SEPARATE tiles (Tile serializes mixed-engine access to one tile).
#     Bias is fused into the Y evictions.
#
# Sharding: data-parallel over batch, 16 batches/core, no collectives.

import numpy as np
import ml_dtypes

import concourse.bass as bass
from concourse import bacc
import concourse.mybir as mybir
import concourse.tile as tile
from concourse.bass_utils import run_bass_kernel_spmd

N_CORES = 8
B = 128
BPC = B // N_CORES      # 16 batches per core
DIM = 384
NPOS = 196
IMG = 14
KS = 27

F32 = mybir.dt.float32
BF16 = mybir.dt.bfloat16
FP16 = mybir.dt.float16
BF16_NP = ml_dtypes.bfloat16

DCH = 3                             # 128-chunks of DIM
VCHUNKS = [(0, 128), (128, 68)]     # token chunks (stage-A contraction)
YW = 3 * NPOS                       # 588 fp16 per batch in the output
XB = 2 * DIM                        # bf16 cols per batch in the x tile
WBW = DCH * DIM + 2 * NPOS + 1      # packed w + bias width
# progressive x-load groups: single batches early (the DMA path runs
# slow for its first ~2us), bigger groups once it has ramped. Batch 0
# rides with mt in the first merged transfer (one semaphore).
XGROUPS = [(2, 1), (3, 1), (4, 4), (8, 8)]
# y stores: taper the tail so the final transfer + receipt is small
YSTORES = [(0, 4), (4, 4), (8, 4), (12, 2), (14, 1), (15, 1)]
# stage-B contraction order: d2 first (vector-evicted, ready earliest)
BDORD = [2, 0, 1]
NWARM = 9


def build_program():
    nc = bacc.Bacc("TRN2", debug=False)

    # x, padded interleaved: [partition, batch, chunk, feature] bf16;
    # chunk 0 = token p, chunk 1 = token 128+p (p<68, else zero pad)
    x_d = nc.dram_tensor("x", [128, BPC, 2, DIM], BF16, kind="ExternalInput")
    # mt packed with batches 0-1's x: one transfer, one semaphore
    mx_d = nc.dram_tensor("mx", [128, 2 * NPOS + 2 * XB], BF16,
                          kind="ExternalInput")
    # packed [w chunks | bias01 | bias2]
    wb_d = nc.dram_tensor("wb", [128, WBW], BF16, kind="ExternalInput")
    # y, e-major fp16: [partition, batch * (e-chunk, u)]
    y_d = nc.dram_tensor("y", [128, BPC * YW], FP16, kind="ExternalOutput")

    with tile.TileContext(nc) as tc:
        with (
            tc.tile_pool(name="const", bufs=1) as const,
            tc.tile_pool(name="work", bufs=2) as work,
            tc.tile_pool(name="psum", bufs=2, space="PSUM") as psum,
        ):
            # ---- constants on sync HWDGE (its queue runs ~5x faster
            # than the scalar-issued ring). mt first: stage A needs
            # only it; then wb; x groups follow ----
            mx_sb = const.tile([128, 2 * NPOS + 2 * XB], BF16)
            nc.sync.dma_start(mx_sb[:, :], mx_d[:, :])
            mt_sb = mx_sb[:, 0:2 * NPOS]
            x0_sb = mx_sb[:, 2 * NPOS:2 * NPOS + 2 * XB]
            # wb rides the scalar-issued HWDGE ring: slower, but fully
            # parallel with the x stream on the sync ring
            wb_sb = const.tile([128, WBW], BF16)
            nc.scalar.dma_start(wb_sb[:, :], wb_d[:, :])
            w_sb = wb_sb[:, 0:DCH * DIM]
            bias01 = wb_sb[:, DCH * DIM:DCH * DIM + 2 * NPOS]
            bias2 = wb_sb[:, DCH * DIM + 2 * NPOS:WBW]

            # ---- x: fully resident, progressive unconditional loads.
            # Order: x group 0 right after mt (unblocks A(0) asap),
            # then wb (needed by B(0)), then the rest of x ----
            x_t = const.tile([128, BPC * XB], BF16, name="xall")
            for gstart, gsize in XGROUPS:
                nc.sync.dma_start(
                    x_t[:, gstart * XB:(gstart + gsize) * XB],
                    x_d[:, gstart:gstart + gsize].rearrange(
                        "p b c d -> p (b c d)"),
                )

            # ---- PE warm-up: dense bf16 matmuls ramp the HAM clock
            # gate to 8/8 while the first x group is in flight ----
            warm_c = nc.const_aps.tensor(1.0, (128, 392), BF16)
            for wi in range(NWARM):
                warm = psum.tile([128, 2 * NPOS], F32, tag="g01",
                                 name=f"warm{wi}")
                nc.tensor.matmul(
                    warm[0:1, :], lhsT=warm_c[:, 0:1], rhs=warm_c,
                    start=True, stop=True,
                )

            # ---- software-pipelined main loop: A(b) then B(b-1) ----
            y_t = None
            gts = {}
            for b in range(BPC + 1):
                if b < BPC:
                    # stage A: G^T_b (d on partitions), tokens = K
                    g01 = psum.tile([128, 2 * NPOS], F32, tag="g01",
                                    name=f"g01_{b}")
                    g2 = psum.tile([128, NPOS], F32, tag="g2",
                                   name=f"g2_{b}")
                    # NOTE: v must stay the inner loop — a group's
                    # start=True clears has_written for the whole bank,
                    # so groups sharing a bank (m0/m1) cannot interleave
                    xsrc = x0_sb if b < 2 else x_t
                    xbase = b * XB
                    for m in range(DCH):
                        dst = (g01[:, m * NPOS:(m + 1) * NPOS] if m < 2
                               else g2[:, :])
                        for v, (v0, vsz) in enumerate(VCHUNKS):
                            xc = xbase + v * DIM + m * 128
                            nc.tensor.matmul(
                                dst,
                                lhsT=xsrc[0:vsz, xc:xc + 128],
                                rhs=mt_sb[0:vsz, v * NPOS:(v + 1) * NPOS],
                                start=(v == 0),
                                stop=(v == 1),
                            )
                    # eviction fp32->bf16 split: scalar d0+d1, vector d2
                    gt = work.tile([128, DCH * NPOS], BF16, tag="gt",
                                   bufs=3, name=f"gt{b}")
                    nc.scalar.copy(gt[:, 0:2 * NPOS], g01[:, :])
                    nc.vector.tensor_copy(gt[:, 2 * NPOS:3 * NPOS],
                                          g2[:, :])
                    gts[b] = gt

                if b >= 1:
                    bb = b - 1        # stage B batch
                    bi = bb % 4
                    if bi == 0:
                        y_t = work.tile([128, 4 * YW], FP16, tag="y",
                                        bufs=2, name=f"y{bb // 4}")
                    gt = gts.pop(bb)

                    # stage B: Y^T_b (e on partitions), d = K, W shared
                    y01 = psum.tile([128, 2 * NPOS], F32, tag="y01",
                                    name=f"y01_{bb}")
                    y2 = psum.tile([128, NPOS], F32, tag="y2",
                                   name=f"y2_{bb}")
                    for e in range(DCH):
                        dst = (y01[:, e * NPOS:(e + 1) * NPOS] if e < 2
                               else y2[:, :])
                        for di, d in enumerate(BDORD):
                            nc.tensor.matmul(
                                dst,
                                lhsT=w_sb[:, d * DIM + e * 128:
                                          d * DIM + e * 128 + 128],
                                rhs=gt[:, d * NPOS:(d + 1) * NPOS],
                                start=(di == 0),
                                stop=(di == DCH - 1),
                            )
                    # eviction + bias fp32->fp16: vector e0+e1, scalar e2
                    nc.vector.tensor_add(
                        y_t[:, bi * YW:bi * YW + 2 * NPOS],
                        y01[:, :], bias01)
                    nc.scalar.add(
                        y_t[:, bi * YW + 2 * NPOS:(bi + 1) * YW],
                        y2[:, :], bias2)
                    for s0, ssz in YSTORES:
                        if bb == s0 + ssz - 1 and bb == BPC - 1:
                            # final batch: store the vector-evicted e0e1
                            # piece and the scalar-evicted e2 piece on
                            # parallel queues so neither receipt waits
                            # on the other eviction
                            nc.sync.dma_start(
                                y_d[:, s0 * YW:s0 * YW + 2 * NPOS],
                                y_t[:, (s0 % 4) * YW:
                                    (s0 % 4) * YW + 2 * NPOS])
                            nc.scalar.dma_start(
                                y_d[:, s0 * YW + 2 * NPOS:
                                    (s0 + ssz) * YW],
                                y_t[:, (s0 % 4) * YW + 2 * NPOS:
                                    (s0 % 4 + ssz) * YW])
                        elif bb == s0 + ssz - 1:
                            nc.sync.dma_start(
                                y_d[:, s0 * YW:(s0 + ssz) * YW],
                                y_t[:, (s0 % 4) * YW:
                                    (s0 % 4 + ssz) * YW])

    nc.compile()
    return nc


_PROGRAM = None


def _get_program():
    global _PROGRAM
    if _PROGRAM is None:
        _PROGRAM = build_program()
    return _PROGRAM


def _host_prep(x, Wv, qk, Wo, bo):
    x = np.asarray(x, dtype=np.float32)
    xb = x.reshape(N_CORES, BPC, NPOS, DIM).astype(BF16_NP)
    # padded interleaved per-core layout: [core, p, b, chunk, d]
    XC = np.zeros((N_CORES, 128, BPC, 2, DIM), dtype=BF16_NP)
    XC[:, :, :, 0, :] = xb[:, :, 0:128].transpose(0, 2, 1, 3)
    XC[:, 0:68, :, 1, :] = xb[:, :, 128:NPOS].transpose(0, 2, 1, 3)
    W = (np.asarray(Wv, np.float32) @ np.asarray(Wo, np.float32)).astype(BF16_NP)
    # MT[(u,v),(p,q)] = qk[13+u-p, 13+v-q]: conv as a 196x196 matmul
    qk2 = np.asarray(qk, np.float32).reshape(KS, KS)
    idx = (KS // 2) + np.arange(IMG)[:, None] - np.arange(IMG)[None, :]
    MT = np.ascontiguousarray(
        qk2[idx[:, None, :, None], idx[None, :, None, :]].reshape(NPOS, NPOS)
    ).astype(BF16_NP)
    mtp = np.zeros((N_CORES, 128, 2 * NPOS + 2 * XB), dtype=BF16_NP)
    mtp[:, :, 0:NPOS] = MT[0:128, :]
    mtp[:, 0:68, NPOS:2 * NPOS] = MT[128:NPOS, :]
    mtp[:, :, 2 * NPOS:] = XC[:, :, 0:2].reshape(N_CORES, 128, 2 * XB)
    bo = np.asarray(bo, np.float32)
    be = bo.reshape(DCH, 128).astype(BF16_NP)    # bias[c][p] = bo[128c+p]
    wb = np.zeros((128, WBW), dtype=BF16_NP)
    # w chunks: wb[p, c*384+e] = W[c*128+p, e]
    wb[:, 0:DCH * DIM] = np.ascontiguousarray(
        W.reshape(DCH, 128, DIM).transpose(1, 0, 2).reshape(128, DCH * DIM))
    for c in range(2):
        wb[:, DCH * DIM + c * NPOS:DCH * DIM + (c + 1) * NPOS] = \
            be[c][:, None]
    wb[:, DCH * DIM + 2 * NPOS] = be[2]
    return XC, mtp, wb


def _unpack_core(y2):
    # y2: [128, BPC*588] fp16 -> (BPC, NPOS, DIM) f32
    a = np.asarray(y2, np.float32).reshape(128, BPC, DCH, NPOS)
    # out[b, u, e=128c+p] = a[p, b, c, u]
    return np.ascontiguousarray(
        a.transpose(1, 3, 2, 0).reshape(BPC, NPOS, DIM))


def _run(x, Wv, qk, Wo, bo, **spmd_kwargs):
    XC, mtp, wb = _host_prep(x, Wv, qk, Wo, bo)
    nc = _get_program()
    in_maps = [
        {"x": XC[c], "mx": mtp[c], "wb": wb}
        for c in range(N_CORES)
    ]
    res = run_bass_kernel_spmd(nc, in_maps, list(range(N_CORES)), **spmd_kwargs)
    y = np.concatenate(
        [_unpack_core(res.results[c]["y"]) for c in range(N_CORES)], axis=0)
    return y, res


def kernel(x, Wv, qk, Wo, bo):
    y, _ = _run(x, Wv, qk, Wo, bo)
    return y


# revision 45
# speedup vs baseline: 1.0688x; 1.0688x over previous
# Trainium2 Bass kernel for nn_Attention_88313117540497.
#
# Reference computation (per batch b of 128):
#   v = x_b @ Wv; conv2d of each channel's 14x14 image with 27x27 qk at
#   padding 13; y = conv_out @ Wo + bo.
#
# Algebra:
#  1. The padded 27x27 conv on 14x14 covers every pixel pair, so it is a
#     dense 196x196 map M shared across batches/channels:
#         y_b = M @ x_b @ (Wv@Wo) + bo,   W = Wv@Wo (384x384).
#  2. PE mapping (out = lhsT.T @ rhs; lhsT stationary, rhs streams N
#     cycles):
#       stage A:  G^T_b = lhsT(X_b).T @ MT    X_b token-major, MT = M^T.
#                 3 d-chunks x 2 v-chunks, N=196.
#       stage B:  Y^T_b = lhsT(W).T @ G^T_b   3 e-chunks x 3 d-chunks,
#                 N=196, full 128x128 array occupancy (FLOP-optimal).
#     All operands bf16: halves DMA, enables Fast Weight Load so the
#     LDWEIGHTS stream hides under the matmul stream.
#  3. Software pipeline: stage A of batch b+1 issues before stage B of
#     batch b, covering the G-eviction latency; a gapless PE stream also
#     keeps the HAM clock gate at 8/8. N=392 bf16 warm-up matmuls ramp
#     HAM while the first x group is in flight.
#  4. x rides in a padded interleaved DRAM layout [128, b, 2, 384] so
#     every transfer is full-partition and contiguous per partition
#     (the natural 196-row layout leaves a slow 68-partition tail).
#  5. PSUM: four single-bank tiles per batch stage (g01/g2/y01/y2,
#     bufs=2 -> exactly 8 banks). The 01 tiles pack two 196-wide matmul
#     outputs contiguously inside one bank, so each eviction is one
#     contiguous 2D op; G and Y evictions are split scalar/vector on
#     SEPARATE tiles (Tile serializes mixed-engine access to one tile).
#     Bias is fused into the Y evictions.
#
# Sharding: data-parallel over batch, 16 batches/core, no collectives.

import numpy as np
import ml_dtypes

import concourse.bass as bass
from concourse import bacc
import concourse.mybir as mybir
import concourse.tile as tile
from concourse.bass_utils import run_bass_kernel_spmd

N_CORES = 8
B = 128
BPC = B // N_CORES      # 16 batches per core
DIM = 384
NPOS = 196
IMG = 14
KS = 27

F32 = mybir.dt.float32
BF16 = mybir.dt.bfloat16
FP16 = mybir.dt.float16
BF16_NP = ml_dtypes.bfloat16

DCH = 3                             # 128-chunks of DIM
VCHUNKS = [(0, 128), (128, 68)]     # token chunks (stage-A contraction)
YW = 3 * NPOS                       # 588 fp16 per batch in the output
XB = 2 * DIM                        # bf16 cols per batch in the x tile
WBW = DCH * DIM + 2 * NPOS + 1      # packed w + bias width
# progressive x-load groups: single batches early (the DMA path runs
# slow for its first ~2us), bigger groups once it has ramped. Batch 0
# rides with mt in the first merged transfer (one semaphore).
XGROUPS = [(1, 1), (2, 1), (3, 1), (4, 4), (8, 8)]
# y stores: taper the tail so the final transfer + receipt is small
YSTORES = [(0, 4), (4, 4), (8, 4), (12, 2), (14, 1), (15, 1)]
# stage-B contraction order: d2 first (vector-evicted, ready earliest)
BDORD = [2, 0, 1]
NWARM = 9


def build_program():
    nc = bacc.Bacc("TRN2", debug=False)

    # x, padded interleaved: [partition, batch, chunk, feature] bf16;
    # chunk 0 = token p, chunk 1 = token 128+p (p<68, else zero pad)
    x_d = nc.dram_tensor("x", [128, BPC, 2, DIM], BF16, kind="ExternalInput")
    # mt packed together with batch 0's x: one transfer, one semaphore
    mx_d = nc.dram_tensor("mx", [128, 2 * NPOS + XB], BF16,
                          kind="ExternalInput")
    # packed [w chunks | bias01 | bias2]
    wb_d = nc.dram_tensor("wb", [128, WBW], BF16, kind="ExternalInput")
    # y, e-major fp16: [partition, batch * (e-chunk, u)]
    y_d = nc.dram_tensor("y", [128, BPC * YW], FP16, kind="ExternalOutput")

    with tile.TileContext(nc) as tc:
        with (
            tc.tile_pool(name="const", bufs=1) as const,
            tc.tile_pool(name="work", bufs=2) as work,
            tc.tile_pool(name="psum", bufs=2, space="PSUM") as psum,
        ):
            # ---- constants on sync HWDGE (its queue runs ~5x faster
            # than the scalar-issued ring). mt first: stage A needs
            # only it; then wb; x groups follow ----
            mx_sb = const.tile([128, 2 * NPOS + XB], BF16)
            nc.sync.dma_start(mx_sb[:, :], mx_d[:, :])
            mt_sb = mx_sb[:, 0:2 * NPOS]
            x0_sb = mx_sb[:, 2 * NPOS:2 * NPOS + XB]
            # wb rides the scalar-issued HWDGE ring: slower, but fully
            # parallel with the x stream on the sync ring
            wb_sb = const.tile([128, WBW], BF16)
            nc.scalar.dma_start(wb_sb[:, :], wb_d[:, :])
            w_sb = wb_sb[:, 0:DCH * DIM]
            bias01 = wb_sb[:, DCH * DIM:DCH * DIM + 2 * NPOS]
            bias2 = wb_sb[:, DCH * DIM + 2 * NPOS:WBW]

            # ---- x: fully resident, progressive unconditional loads.
            # Order: x group 0 right after mt (unblocks A(0) asap),
            # then wb (needed by B(0)), then the rest of x ----
            x_t = const.tile([128, BPC * XB], BF16, name="xall")
            for gstart, gsize in XGROUPS:
                nc.sync.dma_start(
                    x_t[:, gstart * XB:(gstart + gsize) * XB],
                    x_d[:, gstart:gstart + gsize].rearrange(
                        "p b c d -> p (b c d)"),
                )

            # ---- PE warm-up: dense bf16 matmuls ramp the HAM clock
            # gate to 8/8 while the first x group is in flight ----
            warm_c = nc.const_aps.tensor(1.0, (128, 392), BF16)
            for wi in range(NWARM):
                warm = psum.tile([128, 2 * NPOS], F32, tag="g01",
                                 name=f"warm{wi}")
                nc.tensor.matmul(
                    warm[0:1, :], lhsT=warm_c[:, 0:1], rhs=warm_c,
                    start=True, stop=True,
                )

            # ---- software-pipelined main loop: A(b) then B(b-1) ----
            y_t = None
            gts = {}
            for b in range(BPC + 1):
                if b < BPC:
                    # stage A: G^T_b (d on partitions), tokens = K
                    g01 = psum.tile([128, 2 * NPOS], F32, tag="g01",
                                    name=f"g01_{b}")
                    g2 = psum.tile([128, NPOS], F32, tag="g2",
                                   name=f"g2_{b}")
                    # NOTE: v must stay the inner loop — a group's
                    # start=True clears has_written for the whole bank,
                    # so groups sharing a bank (m0/m1) cannot interleave
                    xsrc = x0_sb if b == 0 else x_t
                    xbase = b * XB
                    for m in range(DCH):
                        dst = (g01[:, m * NPOS:(m + 1) * NPOS] if m < 2
                               else g2[:, :])
                        for v, (v0, vsz) in enumerate(VCHUNKS):
                            xc = xbase + v * DIM + m * 128
                            nc.tensor.matmul(
                                dst,
                                lhsT=xsrc[0:vsz, xc:xc + 128],
                                rhs=mt_sb[0:vsz, v * NPOS:(v + 1) * NPOS],
                                start=(v == 0),
                                stop=(v == 1),
                            )
                    # eviction fp32->bf16 split: scalar d0+d1, vector d2
                    gt = work.tile([128, DCH * NPOS], BF16, tag="gt",
                                   bufs=3, name=f"gt{b}")
                    nc.scalar.copy(gt[:, 0:2 * NPOS], g01[:, :])
                    nc.vector.tensor_copy(gt[:, 2 * NPOS:3 * NPOS],
                                          g2[:, :])
                    gts[b] = gt

                if b >= 1:
                    bb = b - 1        # stage B batch
                    bi = bb % 4
                    if bi == 0:
                        y_t = work.tile([128, 4 * YW], FP16, tag="y",
                                        bufs=2, name=f"y{bb // 4}")
                    gt = gts.pop(bb)

                    # stage B: Y^T_b (e on partitions), d = K, W shared
                    y01 = psum.tile([128, 2 * NPOS], F32, tag="y01",
                                    name=f"y01_{bb}")
                    y2 = psum.tile([128, NPOS], F32, tag="y2",
                                   name=f"y2_{bb}")
                    for e in range(DCH):
                        dst = (y01[:, e * NPOS:(e + 1) * NPOS] if e < 2
                               else y2[:, :])
                        for di, d in enumerate(BDORD):
                            nc.tensor.matmul(
                                dst,
                                lhsT=w_sb[:, d * DIM + e * 128:
                                          d * DIM + e * 128 + 128],
                                rhs=gt[:, d * NPOS:(d + 1) * NPOS],
                                start=(di == 0),
                                stop=(di == DCH - 1),
                            )
                    # eviction + bias fp32->fp16: vector e0+e1, scalar e2
                    nc.vector.tensor_add(
                        y_t[:, bi * YW:bi * YW + 2 * NPOS],
                        y01[:, :], bias01)
                    nc.scalar.add(
                        y_t[:, bi * YW + 2 * NPOS:(bi + 1) * YW],
                        y2[:, :], bias2)
                    for s0, ssz in YSTORES:
                        if bb == s0 + ssz - 1 and bb == BPC - 1:
                            # final batch: store the vector-evicted e0e1
                            # piece and the scalar-evicted e2 piece on
                            # parallel queues so neither receipt waits
                            # on the other eviction
                            nc.sync.dma_start(
                                y_d[:, s0 * YW:s0 * YW + 2 * NPOS],
                                y_t[:, (s0 % 4) * YW:
                                    (s0 % 4) * YW + 2 * NPOS])
                            nc.scalar.dma_start(
                                y_d[:, s0 * YW + 2 * NPOS:
                                    (s0 + ssz) * YW],
                                y_t[:, (s0 % 4) * YW + 2 * NPOS:
                                    (s0 % 4 + ssz) * YW])
                        elif bb == s0 + ssz - 1:
                            nc.sync.dma_start(
                                y_d[:, s0 * YW:(s0 + ssz) * YW],
                                y_t[:, (s0 % 4) * YW:
                                    (s0 % 4 + ssz) * YW])

    nc.compile()
    return nc


_PROGRAM = None


def _get_program():
    global _PROGRAM
    if _PROGRAM is None:
        _PROGRAM = build_program()
    return _PROGRAM


def _host_prep(x, Wv, qk, Wo, bo):
    x = np.asarray(x, dtype=np.float32)
    xb = x.reshape(N_CORES, BPC, NPOS, DIM).astype(BF16_NP)
    # padded interleaved per-core layout: [core, p, b, chunk, d]
    XC = np.zeros((N_CORES, 128, BPC, 2, DIM), dtype=BF16_NP)
    XC[:, :, :, 0, :] = xb[:, :, 0:128].transpose(0, 2, 1, 3)
    XC[:, 0:68, :, 1, :] = xb[:, :, 128:NPOS].transpose(0, 2, 1, 3)
    W = (np.asarray(Wv, np.float32) @ np.asarray(Wo, np.float32)).astype(BF16_NP)
    # MT[(u,v),(p,q)] = qk[13+u-p, 13+v-q]: conv as a 196x196 matmul
    qk2 = np.asarray(qk, np.float32).reshape(KS, KS)
    idx = (KS // 2) + np.arange(IMG)[:, None] - np.arange(IMG)[None, :]
    MT = np.ascontiguousarray(
        qk2[idx[:, None, :, None], idx[None, :, None, :]].reshape(NPOS, NPOS)
    ).astype(BF16_NP)
    mtp = np.zeros((N_CORES, 128, 2 * NPOS + XB), dtype=BF16_NP)
    mtp[:, :, 0:NPOS] = MT[0:128, :]
    mtp[:, 0:68, NPOS:2 * NPOS] = MT[128:NPOS, :]
    mtp[:, :, 2 * NPOS:] = XC[:, :, 0].reshape(N_CORES, 128, XB)
    bo = np.asarray(bo, np.float32)
    be = bo.reshape(DCH, 128).astype(BF16_NP)    # bias[c][p] = bo[128c+p]
    wb = np.zeros((128, WBW), dtype=BF16_NP)
    # w chunks: wb[p, c*384+e] = W[c*128+p, e]
    wb[:, 0:DCH * DIM] = np.ascontiguousarray(
        W.reshape(DCH, 128, DIM).transpose(1, 0, 2).reshape(128, DCH * DIM))
    for c in range(2):
        wb[:, DCH * DIM + c * NPOS:DCH * DIM + (c + 1) * NPOS] = \
            be[c][:, None]
    wb[:, DCH * DIM + 2 * NPOS] = be[2]
    return XC, mtp, wb


def _unpack_core(y2):
    # y2: [128, BPC*588] fp16 -> (BPC, NPOS, DIM) f32
    a = np.asarray(y2, np.float32).reshape(128, BPC, DCH, NPOS)
    # out[b, u, e=128c+p] = a[p, b, c, u]
    return np.ascontiguousarray(
        a.transpose(1, 3, 2, 0).reshape(BPC, NPOS, DIM))


def _run(x, Wv, qk, Wo, bo, **spmd_kwargs):
    XC, mtp, wb = _host_prep(x, Wv, qk, Wo, bo)
    nc = _get_program()
    in_maps = [
        {"x": XC[c], "mx": mtp[c], "wb": wb}
        for c in range(N_CORES)
    ]
    res = run_bass_kernel_spmd(nc, in_maps, list(range(N_CORES)), **spmd_kwargs)
    y = np.concatenate(
        [_unpack_core(res.results[c]["y"]) for c in range(N_CORES)], axis=0)
    return y, res


def kernel(x, Wv, qk, Wo, bo):
    y, _ = _run(x, Wv, qk, Wo, bo)
    return y


# revision 46
# speedup vs baseline: 1.1076x; 1.0362x over previous
# Trainium2 Bass kernel for nn_Attention_88313117540497.
#
# Reference computation (per batch b of 128):
#   v = x_b @ Wv; conv2d of each channel's 14x14 image with 27x27 qk at
#   padding 13; y = conv_out @ Wo + bo.
#
# Algebra:
#  1. The padded 27x27 conv on 14x14 covers every pixel pair, so it is a
#     dense 196x196 map M shared across batches/channels:
#         y_b = M @ x_b @ (Wv@Wo) + bo,   W = Wv@Wo (384x384).
#  2. PE mapping (out = lhsT.T @ rhs; lhsT stationary, rhs streams N
#     cycles):
#       stage A:  G^T_b = lhsT(X_b).T @ MT    X_b token-major, MT = M^T.
#                 3 d-chunks x 2 v-chunks, N=196.
#       stage B:  Y^T_b = lhsT(W).T @ G^T_b   3 e-chunks x 3 d-chunks,
#                 N=196, full 128x128 array occupancy (FLOP-optimal).
#     All operands bf16: halves DMA, enables Fast Weight Load so the
#     LDWEIGHTS stream hides under the matmul stream.
#  3. Software pipeline: stage A of batch b+1 issues before stage B of
#     batch b, covering the G-eviction latency; a gapless PE stream also
#     keeps the HAM clock gate at 8/8. N=392 bf16 warm-up matmuls ramp
#     HAM while the first x group is in flight.
#  4. x rides in a padded interleaved DRAM layout [128, b, 2, 384] so
#     every transfer is full-partition and contiguous per partition
#     (the natural 196-row layout leaves a slow 68-partition tail).
#  5. PSUM: four single-bank tiles per batch stage (g01/g2/y01/y2,
#     bufs=2 -> exactly 8 banks). The 01 tiles pack two 196-wide matmul
#     outputs contiguously inside one bank, so each eviction is one
#     contiguous 2D op; G and Y evictions are split scalar/vector on
#     SEPARATE tiles (Tile serializes mixed-engine access to one tile).
#     Bias is fused into the Y evictions.
#
# Sharding: data-parallel over batch, 16 batches/core, no collectives.

import numpy as np
import ml_dtypes

import concourse.bass as bass
from concourse import bacc
import concourse.mybir as mybir
import concourse.tile as tile
from concourse.bass_utils import run_bass_kernel_spmd

N_CORES = 8
B = 128
BPC = B // N_CORES      # 16 batches per core
DIM = 384
NPOS = 196
IMG = 14
KS = 27

F32 = mybir.dt.float32
BF16 = mybir.dt.bfloat16
FP16 = mybir.dt.float16
BF16_NP = ml_dtypes.bfloat16

DCH = 3                             # 128-chunks of DIM
VCHUNKS = [(0, 128), (128, 68)]     # token chunks (stage-A contraction)
YW = 3 * NPOS                       # 588 fp16 per batch in the output
XB = 2 * DIM                        # bf16 cols per batch in the x tile
WBW = DCH * DIM + 2 * NPOS + 1      # packed w + bias width
# progressive x-load groups: single batches early (the DMA path runs
# slow for its first ~2us), bigger groups once it has ramped. Batch 0
# rides with mt in the first merged transfer (one semaphore).
XGROUPS = [(1, 1), (2, 1), (3, 1), (4, 4), (8, 8)]
# y stores: taper the tail so the final transfer + receipt is small
YSTORES = [(0, 4), (4, 4), (8, 4), (12, 2), (14, 1), (15, 1)]
# stage-B contraction order: d2 first (vector-evicted, ready earliest)
BDORD = [2, 0, 1]
NWARM = 11


def build_program():
    nc = bacc.Bacc("TRN2", debug=False)

    # x, padded interleaved: [partition, batch, chunk, feature] bf16;
    # chunk 0 = token p, chunk 1 = token 128+p (p<68, else zero pad)
    x_d = nc.dram_tensor("x", [128, BPC, 2, DIM], BF16, kind="ExternalInput")
    # mt packed together with batch 0's x: one transfer, one semaphore
    mx_d = nc.dram_tensor("mx", [128, 2 * NPOS + XB], BF16,
                          kind="ExternalInput")
    # packed [w chunks | bias01 | bias2]
    wb_d = nc.dram_tensor("wb", [128, WBW], BF16, kind="ExternalInput")
    # y, e-major fp16: [partition, batch * (e-chunk, u)]
    y_d = nc.dram_tensor("y", [128, BPC * YW], FP16, kind="ExternalOutput")

    with tile.TileContext(nc) as tc:
        with (
            tc.tile_pool(name="const", bufs=1) as const,
            tc.tile_pool(name="work", bufs=2) as work,
            tc.tile_pool(name="psum", bufs=2, space="PSUM") as psum,
        ):
            # ---- constants on sync HWDGE (its queue runs ~5x faster
            # than the scalar-issued ring). mt first: stage A needs
            # only it; then wb; x groups follow ----
            mx_sb = const.tile([128, 2 * NPOS + XB], BF16)
            nc.sync.dma_start(mx_sb[:, :], mx_d[:, :])
            mt_sb = mx_sb[:, 0:2 * NPOS]
            x0_sb = mx_sb[:, 2 * NPOS:2 * NPOS + XB]
            # wb rides the scalar-issued HWDGE ring: slower, but fully
            # parallel with the x stream on the sync ring
            wb_sb = const.tile([128, WBW], BF16)
            nc.scalar.dma_start(wb_sb[:, :], wb_d[:, :])
            w_sb = wb_sb[:, 0:DCH * DIM]
            bias01 = wb_sb[:, DCH * DIM:DCH * DIM + 2 * NPOS]
            bias2 = wb_sb[:, DCH * DIM + 2 * NPOS:WBW]

            # ---- x: fully resident, progressive unconditional loads.
            # Order: x group 0 right after mt (unblocks A(0) asap),
            # then wb (needed by B(0)), then the rest of x ----
            x_t = const.tile([128, BPC * XB], BF16, name="xall")
            for gstart, gsize in XGROUPS:
                nc.sync.dma_start(
                    x_t[:, gstart * XB:(gstart + gsize) * XB],
                    x_d[:, gstart:gstart + gsize].rearrange(
                        "p b c d -> p (b c d)"),
                )

            # ---- PE warm-up: dense bf16 matmuls ramp the HAM clock
            # gate to 8/8 while the first x group is in flight ----
            warm_c = nc.const_aps.tensor(1.0, (128, 392), BF16)
            for wi in range(NWARM):
                warm = psum.tile([128, 2 * NPOS], F32, tag="g01",
                                 name=f"warm{wi}")
                nc.tensor.matmul(
                    warm[0:1, :], lhsT=warm_c[:, 0:1], rhs=warm_c,
                    start=True, stop=True,
                )

            # ---- software-pipelined main loop: A(b) then B(b-1) ----
            y_t = None
            gts = {}
            for b in range(BPC + 1):
                if b < BPC:
                    # stage A: G^T_b (d on partitions), tokens = K
                    g01 = psum.tile([128, 2 * NPOS], F32, tag="g01",
                                    name=f"g01_{b}")
                    g2 = psum.tile([128, NPOS], F32, tag="g2",
                                   name=f"g2_{b}")
                    # NOTE: v must stay the inner loop — a group's
                    # start=True clears has_written for the whole bank,
                    # so groups sharing a bank (m0/m1) cannot interleave
                    xsrc = x0_sb if b == 0 else x_t
                    xbase = b * XB
                    for m in range(DCH):
                        dst = (g01[:, m * NPOS:(m + 1) * NPOS] if m < 2
                               else g2[:, :])
                        for v, (v0, vsz) in enumerate(VCHUNKS):
                            xc = xbase + v * DIM + m * 128
                            nc.tensor.matmul(
                                dst,
                                lhsT=xsrc[0:vsz, xc:xc + 128],
                                rhs=mt_sb[0:vsz, v * NPOS:(v + 1) * NPOS],
                                start=(v == 0),
                                stop=(v == 1),
                            )
                    # eviction fp32->bf16 split: scalar d0+d1, vector d2
                    gt = work.tile([128, DCH * NPOS], BF16, tag="gt",
                                   bufs=3, name=f"gt{b}")
                    nc.scalar.copy(gt[:, 0:2 * NPOS], g01[:, :])
                    nc.vector.tensor_copy(gt[:, 2 * NPOS:3 * NPOS],
                                          g2[:, :])
                    gts[b] = gt

                if b >= 1:
                    bb = b - 1        # stage B batch
                    bi = bb % 4
                    if bi == 0:
                        y_t = work.tile([128, 4 * YW], FP16, tag="y",
                                        bufs=2, name=f"y{bb // 4}")
                    gt = gts.pop(bb)

                    # stage B: Y^T_b (e on partitions), d = K, W shared
                    y01 = psum.tile([128, 2 * NPOS], F32, tag="y01",
                                    name=f"y01_{bb}")
                    y2 = psum.tile([128, NPOS], F32, tag="y2",
                                   name=f"y2_{bb}")
                    for e in range(DCH):
                        dst = (y01[:, e * NPOS:(e + 1) * NPOS] if e < 2
                               else y2[:, :])
                        for di, d in enumerate(BDORD):
                            nc.tensor.matmul(
                                dst,
                                lhsT=w_sb[:, d * DIM + e * 128:
                                          d * DIM + e * 128 + 128],
                                rhs=gt[:, d * NPOS:(d + 1) * NPOS],
                                start=(di == 0),
                                stop=(di == DCH - 1),
                            )
                    # eviction + bias fp32->fp16: vector e0+e1, scalar e2
                    nc.vector.tensor_add(
                        y_t[:, bi * YW:bi * YW + 2 * NPOS],
                        y01[:, :], bias01)
                    nc.scalar.add(
                        y_t[:, bi * YW + 2 * NPOS:(bi + 1) * YW],
                        y2[:, :], bias2)
                    for s0, ssz in YSTORES:
                        if bb == s0 + ssz - 1 and bb == BPC - 1:
                            # final batch: store the vector-evicted e0e1
                            # piece and the scalar-evicted e2 piece on
                            # parallel queues so neither receipt waits
                            # on the other eviction
                            nc.sync.dma_start(
                                y_d[:, s0 * YW:s0 * YW + 2 * NPOS],
                                y_t[:, (s0 % 4) * YW:
                                    (s0 % 4) * YW + 2 * NPOS])
                            nc.scalar.dma_start(
                                y_d[:, s0 * YW + 2 * NPOS:
                                    (s0 + ssz) * YW],
                                y_t[:, (s0 % 4) * YW + 2 * NPOS:
                                    (s0 % 4 + ssz) * YW])
                        elif bb == s0 + ssz - 1:
                            nc.sync.dma_start(
                                y_d[:, s0 * YW:(s0 + ssz) * YW],
                                y_t[:, (s0 % 4) * YW:
                                    (s0 % 4 + ssz) * YW])

    nc.compile()
    return nc


_PROGRAM = None


def _get_program():
    global _PROGRAM
    if _PROGRAM is None:
        _PROGRAM = build_program()
    return _PROGRAM


def _host_prep(x, Wv, qk, Wo, bo):
    x = np.asarray(x, dtype=np.float32)
    xb = x.reshape(N_CORES, BPC, NPOS, DIM).astype(BF16_NP)
    # padded interleaved per-core layout: [core, p, b, chunk, d]
    XC = np.zeros((N_CORES, 128, BPC, 2, DIM), dtype=BF16_NP)
    XC[:, :, :, 0, :] = xb[:, :, 0:128].transpose(0, 2, 1, 3)
    XC[:, 0:68, :, 1, :] = xb[:, :, 128:NPOS].transpose(0, 2, 1, 3)
    W = (np.asarray(Wv, np.float32) @ np.asarray(Wo, np.float32)).astype(BF16_NP)
    # MT[(u,v),(p,q)] = qk[13+u-p, 13+v-q]: conv as a 196x196 matmul
    qk2 = np.asarray(qk, np.float32).reshape(KS, KS)
    idx = (KS // 2) + np.arange(IMG)[:, None] - np.arange(IMG)[None, :]
    MT = np.ascontiguousarray(
        qk2[idx[:, None, :, None], idx[None, :, None, :]].reshape(NPOS, NPOS)
    ).astype(BF16_NP)
    mtp = np.zeros((N_CORES, 128, 2 * NPOS + XB), dtype=BF16_NP)
    mtp[:, :, 0:NPOS] = MT[0:128, :]
    mtp[:, 0:68, NPOS:2 * NPOS] = MT[128:NPOS, :]
    mtp[:, :, 2 * NPOS:] = XC[:, :, 0].reshape(N_CORES, 128, XB)
    bo = np.asarray(bo, np.float32)
    be = bo.reshape(DCH, 128).astype(BF16_NP)    # bias[c][p] = bo[128c+p]
    wb = np.zeros((128, WBW), dtype=BF16_NP)
    # w chunks: wb[p, c*384+e] = W[c*128+p, e]
    wb[:, 0:DCH * DIM] = np.ascontiguousarray(
        W.reshape(DCH, 128, DIM).transpose(1, 0, 2).reshape(128, DCH * DIM))
    for c in range(2):
        wb[:, DCH * DIM + c * NPOS:DCH * DIM + (c + 1) * NPOS] = \
            be[c][:, None]
    wb[:, DCH * DIM + 2 * NPOS] = be[2]
    return XC, mtp, wb


def _unpack_core(y2):
    # y2: [128, BPC*588] fp16 -> (BPC, NPOS, DIM) f32
    a = np.asarray(y2, np.float32).reshape(128, BPC, DCH, NPOS)
    # out[b, u, e=128c+p] = a[p, b, c, u]
    return np.ascontiguousarray(
        a.transpose(1, 3, 2, 0).reshape(BPC, NPOS, DIM))


def _run(x, Wv, qk, Wo, bo, **spmd_kwargs):
    XC, mtp, wb = _host_prep(x, Wv, qk, Wo, bo)
    nc = _get_program()
    in_maps = [
        {"x": XC[c], "mx": mtp[c], "wb": wb}
        for c in range(N_CORES)
    ]
    res = run_bass_kernel_spmd(nc, in_maps, list(range(N_CORES)), **spmd_kwargs)
    y = np.concatenate(
        [_unpack_core(res.results[c]["y"]) for c in range(N_CORES)], axis=0)
    return y, res


def kernel(x, Wv, qk, Wo, bo):
    y, _ = _run(x, Wv, qk, Wo, bo)
    return y
